# revision 25
# baseline (speedup 1.0000x reference)
"""Trainium2 Bass kernel for nn_Attention_40020505264416.

Reference computation (B=4, H=16, N=1024, C=64, D=H*C=1024):
    scores = einsum('bhnc,bhmc->bhnm', q, k) * C**-0.5
    attn   = pe + softmax(scores, axis=-1)          # post-softmax bias
    ctx    = einsum('bhnm,bhmc->bhnc', attn, v)
    x      = ctx.transpose(0,2,1,3).reshape(B, N, D)
    out    = silu(x @ w1 + b1) @ w2 + b2

Distribution: pure data-parallel over query rows (N sharded 8-way, 128
rows per core).  No inter-core communication.  Host-side layouts:

  qk  [H,B/2,128,N+NS] fp8   kT|qT packed, two batches stacked on the
                             partition axis (b even rows 0:64, b odd
                             64:128) -> the two batches' S^T matmuls
                             target disjoint PE row-groups and run
                             CONCURRENTLY (tile_position auto-derived
                             from base_partition).
  pv  [H,N,NS+B*(C+1)] bf16  peT slices | v with ones column (the AV
                             matmul emits the softmax denominator as
                             psum column 64 for free)
  w*, b*                     MLP weights, natural layout

Steady state is ACT(exp)-bound: per batch-pair the ACT does two
[128,1024] exps (~2.2us) while the PE does the 16 interleaved S^T
matmuls (concurrent row-halves), the previous pair's AV + fixups, and
an eighth of the head's pe@v.  S^T psum tiles are bf16 (1 bank each) so
four tiles = two batch-pairs pipeline in 4 of the 8 psum banks.

DMA: everything rides the HWDGE rings.  sync: qk + pv + w1 (streamed
over heads 6..13) + w2 (MLP start) + outputs; scalar: tiny init loads.
Explicit 2-head prefetch keeps the PE/ACT pipe fed; q/k in fp8 keeps
the attention-window DMA demand at ~245 GB/s (<358 limit).  fp8 scores
only perturb softmax weights ~5% rel; the softmax branch contributes
~0.2% of x's variance (pe@v dominates), so the end-to-end error stays
at the bf16 baseline level.
"""

import os
import sys

for _p in ("/opt/trn_rl_repo",):
    if os.path.isdir(_p) and _p not in sys.path:
        sys.path.insert(0, _p)

import numpy as np

import concourse.bass as bass
import concourse.mybir as mybir
import concourse.tile as tile
from concourse import bacc
from concourse.bass_utils import run_bass_kernel_spmd

B, H, N, C = 4, 16, 1024, 64
D = H * C
NCORES = 8
NS = N // NCORES          # query rows per core
J = N // 128              # key chunks of 128
SCALE = C ** -0.5
PF = 3                    # DMA prefetch depth (heads)

PVW = NS + B * (C + 1)    # packed peT|v' row width
JH = J // 2               # half of the key chunks (one 1-bank psum tile)
F32 = mybir.dt.float32
BF16 = mybir.dt.bfloat16
F8 = mybir.dt.float8e4


def build_program():
    nc = bacc.Bacc(None, debug=False)

    # k^T and q^T packed in one tensor, two batches stacked on the
    # partition axis: [h, b//2, (b%2)*C+c, 0:N]=kT, [.., N:N+NS]=qT
    qk_d = nc.dram_tensor("qk", [H, B // 2, 2 * C, N + NS], F8,
                          kind="ExternalInput")
    # pe^T and v' packed per head: [h, m, 0:NS]=peT(q), [h, m, NS:]=v'(b,c+1)
    pv_d = nc.dram_tensor("pv", [H, N, PVW], BF16, kind="ExternalInput")
    idm_d = nc.dram_tensor("idm", [128, 128], BF16, kind="ExternalInput")
    w1_d = nc.dram_tensor("w1s", [D, D], BF16, kind="ExternalInput")
    b1_d = nc.dram_tensor("b1s", [D], F32, kind="ExternalInput")
    w2_d = nc.dram_tensor("w2s", [D, D], BF16, kind="ExternalInput")
    b2_d = nc.dram_tensor("b2s", [D], BF16, kind="ExternalInput")
    out_d = nc.dram_tensor("out", [B, NS, D], F32, kind="ExternalOutput")

    with tile.TileContext(nc) as tc:
        from contextlib import ExitStack

        with ExitStack() as ctx:
            const = ctx.enter_context(tc.tile_pool(name="const", bufs=1))

            ident = const.tile([128, 128], BF16, tag="ident")
            nc.scalar.dma_start(ident[:], idm_d[:])
            ones1 = const.tile([1, 128], BF16, tag="ones1")
            nc.vector.memset(ones1[:], 1.0)

            w1_s = const.tile([128, D // 128, D], BF16, tag="w1s")
            w2_s = const.tile([128, D // 128, D], BF16, tag="w2s")
            w1_r = w1_d.rearrange("(i p) o -> p i o", p=128)
            w2_r = w2_d.rearrange("(i p) o -> p i o", p=128)
            b1_s = const.tile([128, D // 128], F32, tag="b1s")
            nc.scalar.dma_start(b1_s[:], b1_d.rearrange("(o p) -> p o", p=128))
            b2_s = const.tile([1, D], BF16, tag="b2s")
            nc.scalar.dma_start(b2_s[:], b2_d.rearrange("(x d) -> x d", x=1))

            # warm-up fodder: dependency-free matmuls ramp the PE clock
            # while the first DMAs land; a dummy Exp loads the ACT table.
            warm_w = const.tile([128, 128], BF16, tag="warmw", name="warm_w")
            nc.vector.memset(warm_w[:], 0.0)
            warm_act = const.tile([128, 16], F32, tag="warma", name="warm_act")

            # Attention output, natural layout [q, d] per batch.
            x_nat = [const.tile([NS, H, C], BF16, tag=f"xnat{b}",
                                name=f"xnat{b}")
                     for b in range(B)]
            # x^T chunks [d-in-chunk, chunk, b, q] and hdn^T chunks.
            xT = const.tile([128, D // 128, B, NS], BF16, tag="xT")
            hdnT = const.tile([128, D // 128, B, NS], BF16, tag="hdnT")

            # ---------------- attention ----------------
            with ExitStack() as attn_ctx:
                pool_pe = attn_ctx.enter_context(
                    tc.tile_pool(name="pe", bufs=4))
                pool_v = attn_ctx.enter_context(
                    tc.tile_pool(name="v", bufs=PF + 2))
                pool_k = attn_ctx.enter_context(
                    tc.tile_pool(name="k", bufs=2 * (PF + 1)))
                pool_e = attn_ctx.enter_context(
                    tc.tile_pool(name="e", bufs=4))
                pool_r = attn_ctx.enter_context(
                    tc.tile_pool(name="r", bufs=4))
                # S^T tiles are [128, J, 128] fp32 = 4KB = two psum banks
                # each; one batch-pair in flight (the next pair's matmuls
                # start as soon as the first exp frees its banks).
                psum_s = attn_ctx.enter_context(
                    tc.tile_pool(name="ps", bufs=2, space="PSUM"))
                psum_pe = attn_ctx.enter_context(
                    tc.tile_pool(name="ppe", bufs=1, space="PSUM"))
                psum_av = attn_ctx.enter_context(
                    tc.tile_pool(name="pav", bufs=2, space="PSUM"))
                psum_w = attn_ctx.enter_context(
                    tc.tile_pool(name="pw", bufs=1, space="PSUM"))

                # ACT exp table load, off the critical path.
                nc.scalar.activation(
                    warm_act[:], warm_w[:, 0:16],
                    mybir.ActivationFunctionType.Exp)

                # ~2.6us of dependency-free matmuls to ramp the PE clock;
                # wt stays live as the filler target through attention.
                wt = psum_w.tile([128, 128], F32, tag="wt", name="warm_t")
                for _ in range(24):
                    nc.tensor.matmul(wt[:], warm_w[:], warm_w[:],
                                     start=True, stop=True)

                heads = {}

                def issue_head_dmas(h):
                    if h >= H:
                        return
                    qk_t = [None, None]
                    for bp in range(2):
                        t = pool_k.tile([2 * C, N + NS], F8, tag="kT",
                                        name=f"qk{h}_{bp}")
                        nc.sync.dma_start(t[:], qk_d[h, bp])
                        qk_t[bp] = t
                    pv_t = pool_v.tile([128, J, PVW], BF16, tag="vp",
                                       name=f"pv{h}")
                    nc.sync.dma_start(
                        pv_t[:], pv_d[h].rearrange("(j p) x -> p j x", p=128))
                    if 4 <= h < 4 + D // 128:
                        nc.sync.dma_start(w1_s[:, h - 4, :], w1_r[:, h - 4, :])
                    if 8 <= h < 8 + D // 128:
                        nc.sync.dma_start(w2_s[:, h - 8, :], w2_r[:, h - 8, :])
                    heads[h] = (qk_t, pv_t)

                for h in range(PF):
                    issue_head_dmas(h)

                pe4_ps = [None]

                def emit_pe4(h, parts=(0, 1)):
                    """pe @ v for all 4 batches of head h (q-major,
                    batches concatenated on the free axis).  Emitted in
                    two 4-chunk halves, one per preceding batch-pair, so
                    the PE load is even and the next S^T is never stuck
                    behind a long pe4 burst.  Returns pe4_sb after the
                    last part, else None."""
                    _, vp_t = heads[h]
                    vp4 = vp_t[:, :, NS:].rearrange(
                        "p j (b c) -> p j b c", b=B)
                    peT_t = vp_t[:, :, 0:NS]
                    if 0 in parts:
                        pe4_ps[0] = psum_pe.tile([NS, B, C + 1], F32,
                                                 tag="pe4", name="pe4")
                    pe4 = pe4_ps[0]
                    for part in parts:
                        for j in range(part * J // 2, (part + 1) * J // 2):
                            nc.tensor.matmul(
                                pe4[:], peT_t[:, j, :], vp4[:, j, :, :],
                                start=(j == 0), stop=(j == J - 1))
                    if 1 not in parts:
                        return None
                    pe4_sb = pool_pe.tile([NS, B, C + 1], F32, tag="pe4sb",
                                          name="pe4_sb")
                    # stage in SBUF: DVE may read only one PSUM input
                    nc.vector.tensor_copy(pe4_sb[:], pe4[:])
                    return pe4_sb

                def do_av(prev):
                    """AV matmuls + normalization fixup for a finished
                    batch-pair (pipelined one pair late so the PE never
                    waits on the current pair's exps)."""
                    h, bp, exps, vp_t, pe4_sb = prev
                    vp4 = vp_t[:, :, NS:].rearrange(
                        "p j (b c) -> p j b c", b=B)
                    for i in range(2):
                        b = 2 * bp + i
                        av = psum_av.tile([NS, C + 1], F32, tag="av",
                                          name="av")
                        for j in range(J):
                            nc.tensor.matmul(
                                av[:], exps[i][:, j, :], vp4[:, j, b, :],
                                start=(j == 0), stop=(j == J - 1))
                        recip = pool_r.tile([NS, 1], F32, tag="recip",
                                            name="recip")
                        nc.vector.reciprocal(recip[:], av[:, C:C + 1])
                        # x = ctx_exp/den + ctx_pe
                        nc.vector.scalar_tensor_tensor(
                            out=x_nat[b][:, h, :],
                            in0=av[:, 0:C],
                            scalar=recip[:, 0:1],
                            in1=pe4_sb[:, b, 0:C],
                            op0=mybir.AluOpType.mult,
                            op1=mybir.AluOpType.add)
                        if h % 2 == 1:
                            # both heads of chunk h//2 are in x_nat[b]:
                            # transpose to xT inline
                            t = h // 2
                            pt = psum_av.tile([128, NS], BF16, tag="av",
                                              name="pt")
                            nc.tensor.transpose(
                                pt[:], x_nat[b][:, h - 1:h + 1, :], ident[:])
                            nc.vector.tensor_copy(xT[:, t, b, :], pt[:])

                prev = None
                pe4_sb_cur = None       # pe4_sb for head h (consumed by h's
                pe4_sb_next = None      # do_av calls); next = head h+1
                for h in range(H):
                    qk_t, vp_t = heads[h]
                    if h == 0:
                        pass  # pe4(0) emitted inside (0, bp0) below
                    for bp in range(2):
                        qk = qk_t[bp]
                        exps = [None, None]
                        st = [None, None]
                        # interleaved S^T: even batch on PE rows 0:64,
                        # odd batch on rows 64:128 -> concurrent halves
                        for i in range(2):
                            st[i] = psum_s.tile([128, J, NS], F32, tag="st",
                                                name=f"st{i}")
                        for j in range(J):
                            for i in range(2):
                                s = i * C
                                nc.tensor.matmul(
                                    st[i][:, j, :],
                                    qk[s:s + C, j * 128:(j + 1) * 128],
                                    qk[s:s + C, N:],
                                    start=True, stop=True)
                        # dependency-free fillers: keep the PE activity
                        # monitor fed so the clock stays at 8/8 (the
                        # packed S^T halves array duty; HAM re-throttles
                        # an under-occupied PE even with no real gaps).
                        # LDWEIGHTS-only: no psum write, so the fillers
                        # don't contend with the ACT/DVE psum reads.
                        for _ in range(3):
                            nc.tensor.ldweights(warm_w[:])
                        for i in range(2):
                            e = pool_e.tile([128, J, NS], BF16, tag="expS",
                                            name=f"expS{i}")
                            nc.scalar.activation(
                                e[:], st[i][:],
                                mybir.ActivationFunctionType.Exp,
                                scale=SCALE)
                            exps[i] = e

                        if prev is not None:
                            do_av(prev)
                        if h == 0 and bp == 0:
                            pe4_sb_cur = emit_pe4(0)
                        if h + 1 < H:
                            # pipeline next head's pe@v a half-head early,
                            # one 4-chunk part per batch-pair
                            r = emit_pe4(h + 1, parts=(bp,))
                            if bp == 1:
                                pe4_sb_next = r
                        prev = (h, bp, exps, vp_t, pe4_sb_cur)
                    pe4_sb_cur = pe4_sb_next
                    issue_head_dmas(h + PF)
                do_av(prev)

            # ---------------- MLP ----------------
            with ExitStack() as mlp_ctx:
                psum_h1 = mlp_ctx.enter_context(
                    tc.tile_pool(name="ph1", bufs=3, space="PSUM"))
                psum_y = mlp_ctx.enter_context(
                    tc.tile_pool(name="py", bufs=2, space="PSUM"))

                # b2 broadcast to all 128 row-partitions (PE outer
                # product with ones), staged to SBUF for the fc2 bias add
                b2b = const.tile([128, D], BF16, tag="b2b", name="b2b")
                for nn in range(2):
                    yb = psum_y.tile([128, 512], F32, tag="y", name="b2bp")
                    nc.tensor.matmul(
                        yb[:], ones1[:1, :], b2_s[:1, nn * 512:(nn + 1) * 512],
                        start=True, stop=True)
                    nc.vector.tensor_copy(b2b[:, nn * 512:(nn + 1) * 512],
                                          yb[:])

                # fc1: hdn^T[do, rows] = sum_i w1[i]^T.T @ xT[i]
                pool_sg = mlp_ctx.enter_context(tc.tile_pool(name="sg",
                                                             bufs=3))
                for o in range(D // 128):
                    h1 = psum_h1.tile([128, B, NS], F32, tag="h1")
                    for i in range(D // 128):
                        nc.tensor.matmul(
                            h1[:], w1_s[:, i, o * 128:(o + 1) * 128],
                            xT[:, i, :, :],
                            start=(i == 0), stop=(i == D // 128 - 1))
                    # silu(z) = z * sigmoid(z), z = h1 + b1
                    sg = pool_sg.tile([128, B, NS], F32, tag="sg")
                    nc.scalar.activation(
                        sg[:], h1[:],
                        mybir.ActivationFunctionType.Sigmoid,
                        bias=b1_s[:, o:o + 1])
                    nc.vector.scalar_tensor_tensor(
                        out=hdnT[:, o, :, :],
                        in0=h1[:],
                        scalar=b1_s[:, o:o + 1],
                        in1=sg[:],
                        op0=mybir.AluOpType.add,
                        op1=mybir.AluOpType.mult)

                # fc2: y[rows, do] = sum_i hdnT[i].T @ w2[i]  (+ b2)
                pool_o = mlp_ctx.enter_context(tc.tile_pool(name="o",
                                                            bufs=3))
                for t in range(B):
                    for nn in range(2):
                        y = psum_y.tile([128, 512], F32, tag="y")
                        for i in range(D // 128):
                            nc.tensor.matmul(
                                y[:], hdnT[:, i, t, :],
                                w2_s[:, i, nn * 512:(nn + 1) * 512],
                                start=(i == 0), stop=(i == D // 128 - 1))
                        y_sb = pool_o.tile([128, 512], F32, tag="ysb")
                        nc.vector.tensor_tensor(
                            out=y_sb[:], in0=y[:],
                            in1=b2b[:, nn * 512:(nn + 1) * 512],
                            op=mybir.AluOpType.add)
                        nc.sync.dma_start(
                            out_d[t, :, nn * 512:(nn + 1) * 512], y_sb[:])

    nc.compile()
    return nc


_PROG = None


def _get_prog():
    global _PROG
    if _PROG is None:
        _PROG = build_program()
    return _PROG


def make_in_maps(q, k, v, pe, w1, b1, w2, b2):
    import ml_dtypes
    bf = ml_dtypes.bfloat16
    f8 = ml_dtypes.float8_e4m3
    # [b,h,n,c] -> [h, b//2, (b%2)*C+c, n]
    qT = np.transpose(q, (1, 0, 3, 2)).reshape(H, B // 2, 2 * C, N)
    kT = np.transpose(k, (1, 0, 3, 2)).reshape(H, B // 2, 2 * C, N)
    vp = np.concatenate([v, np.ones((B, H, N, 1), v.dtype)], axis=-1)
    vp = np.transpose(vp, (1, 2, 0, 3)).reshape(H, N, B * (C + 1)).astype(bf)
    peT = np.transpose(pe[0], (0, 2, 1)).astype(bf)
    w1c = np.ascontiguousarray(w1).astype(bf)
    w2c = np.ascontiguousarray(w2).astype(bf)
    b1f = np.ascontiguousarray(b1).astype(np.float32)
    b2c = np.ascontiguousarray(b2).astype(bf)
    idm = np.eye(128, dtype=np.float32).astype(bf)

    in_maps = []
    for r in range(NCORES):
        sl = slice(r * NS, (r + 1) * NS)
        # kT is full N (not sharded); qT carries this core's q rows
        qk = np.ascontiguousarray(
            np.concatenate([kT, qT[:, :, :, sl]], axis=-1)).astype(f8)
        pv = np.ascontiguousarray(
            np.concatenate([peT[:, :, sl], vp], axis=-1))
        in_maps.append({
            "qk": qk,
            "pv": pv,
            "idm": idm,
            "w1s": w1c,
            "b1s": b1f,
            "w2s": w2c,
            "b2s": b2c,
        })
    return in_maps


def assemble(results):
    out = np.empty((B, N, D), np.float32)
    for r in range(NCORES):
        out[:, r * NS:(r + 1) * NS, :] = results[r]["out"]
    return out


def kernel(q, k, v, pe, w1, b1, w2, b2):
    nc = _get_prog()
    in_maps = make_in_maps(q, k, v, pe, w1, b1, w2, b2)
    res = run_bass_kernel_spmd(nc, in_maps, core_ids=list(range(NCORES)))
    return assemble(res.results)


# revision 27
# speedup vs baseline: 1.4552x; 1.4552x over previous
"""Trainium2 Bass kernel for nn_Attention_40020505264416.

Reference computation (B=4, H=16, N=1024, C=64, D=H*C=1024):
    scores = einsum('bhnc,bhmc->bhnm', q, k) * C**-0.5
    attn   = pe + softmax(scores, axis=-1)          # post-softmax bias
    ctx    = einsum('bhnm,bhmc->bhnc', attn, v)
    x      = ctx.transpose(0,2,1,3).reshape(B, N, D)
    out    = silu(x @ w1 + b1) @ w2 + b2

Distribution: pure data-parallel over query rows (N sharded 8-way, 128
rows per core).  No inter-core communication.  Host-side layouts:

  qk  [H,B/2,128,N+NS] fp8   kT|qT packed, two batches stacked on the
                             partition axis (b even rows 0:64, b odd
                             64:128) -> the two batches' S^T matmuls
                             target disjoint PE row-groups and run
                             CONCURRENTLY (tile_position auto-derived
                             from base_partition).
  pv  [H,N,NS+B*(C+1)] bf16  peT slices | v with ones column (the AV
                             matmul emits the softmax denominator as
                             psum column 64 for free)
  w*, b*                     MLP weights, natural layout

Steady state is ACT(exp)-bound: per batch-pair the ACT does two
[128,1024] exps (~2.2us) while the PE does the 16 interleaved S^T
matmuls (concurrent row-halves), the previous pair's AV + fixups, and
an eighth of the head's pe@v.  S^T psum tiles are bf16 (1 bank each) so
four tiles = two batch-pairs pipeline in 4 of the 8 psum banks.

DMA: everything rides the HWDGE rings.  sync: qk + pv + w1 (streamed
over heads 6..13) + w2 (MLP start) + outputs; scalar: tiny init loads.
Explicit 2-head prefetch keeps the PE/ACT pipe fed; q/k in fp8 keeps
the attention-window DMA demand at ~245 GB/s (<358 limit).  fp8 scores
only perturb softmax weights ~5% rel; the softmax branch contributes
~0.2% of x's variance (pe@v dominates), so the end-to-end error stays
at the bf16 baseline level.
"""

import os
import sys

for _p in ("/opt/trn_rl_repo",):
    if os.path.isdir(_p) and _p not in sys.path:
        sys.path.insert(0, _p)

import numpy as np

import concourse.bass as bass
import concourse.mybir as mybir
import concourse.tile as tile
from concourse import bacc
from concourse.bass_utils import run_bass_kernel_spmd

B, H, N, C = 4, 16, 1024, 64
D = H * C
NCORES = 8
NS = N // NCORES          # query rows per core
J = N // 128              # key chunks of 128
SCALE = C ** -0.5
PF = 2                    # DMA prefetch depth (heads)

PVW = NS + B * (C + 1)    # packed peT|v' row width
JH = J // 2               # half of the key chunks (one 1-bank psum tile)
F32 = mybir.dt.float32
BF16 = mybir.dt.bfloat16
F8 = mybir.dt.float8e4


def build_program():
    nc = bacc.Bacc(None, debug=False)

    # k^T and q^T packed in one tensor, two batches stacked on the
    # partition axis: [h, b//2, (b%2)*C+c, 0:N]=kT, [.., N:N+NS]=qT
    qk_d = nc.dram_tensor("qk", [H, B // 2, 2 * C, N + NS], F8,
                          kind="ExternalInput")
    # pe^T and v' packed per head: [h, m, 0:NS]=peT(q), [h, m, NS:]=v'(b,c+1)
    pv_d = nc.dram_tensor("pv", [H, N, PVW], BF16, kind="ExternalInput")
    idm_d = nc.dram_tensor("idm", [128, 128], BF16, kind="ExternalInput")
    w1_d = nc.dram_tensor("w1s", [D, D], BF16, kind="ExternalInput")
    b1_d = nc.dram_tensor("b1s", [D], F32, kind="ExternalInput")
    w2_d = nc.dram_tensor("w2s", [D, D], BF16, kind="ExternalInput")
    b2_d = nc.dram_tensor("b2s", [D], BF16, kind="ExternalInput")
    out_d = nc.dram_tensor("out", [B, NS, D], F32, kind="ExternalOutput")

    with tile.TileContext(nc) as tc:
        from contextlib import ExitStack

        with ExitStack() as ctx:
            const = ctx.enter_context(tc.tile_pool(name="const", bufs=1))

            ident = const.tile([128, 128], BF16, tag="ident")
            nc.scalar.dma_start(ident[:], idm_d[:])
            ones1 = const.tile([1, 128], BF16, tag="ones1")
            nc.vector.memset(ones1[:], 1.0)

            w1_s = const.tile([128, D // 128, D], BF16, tag="w1s")
            w2_s = const.tile([128, D // 128, D], BF16, tag="w2s")
            w1_r = w1_d.rearrange("(i p) o -> p i o", p=128)
            w2_r = w2_d.rearrange("(i p) o -> p i o", p=128)
            b1_s = const.tile([128, D // 128], F32, tag="b1s")
            nc.scalar.dma_start(b1_s[:], b1_d.rearrange("(o p) -> p o", p=128))
            b2_s = const.tile([1, D], BF16, tag="b2s")
            nc.scalar.dma_start(b2_s[:], b2_d.rearrange("(x d) -> x d", x=1))

            # warm-up fodder: dependency-free matmuls ramp the PE clock
            # while the first DMAs land; a dummy Exp loads the ACT table.
            warm_w = const.tile([128, 128], BF16, tag="warmw", name="warm_w")
            nc.vector.memset(warm_w[:], 0.0)
            warm_act = const.tile([128, 16], F32, tag="warma", name="warm_act")

            # Attention output, natural layout [q, d] per batch.
            x_nat = [const.tile([NS, H, C], BF16, tag=f"xnat{b}",
                                name=f"xnat{b}")
                     for b in range(B)]
            # x^T chunks [d-in-chunk, chunk, b, q] and hdn^T chunks.
            xT = const.tile([128, D // 128, B, NS], BF16, tag="xT")
            hdnT = const.tile([128, D // 128, B, NS], BF16, tag="hdnT")

            # ---------------- attention ----------------
            with ExitStack() as attn_ctx:
                pool_pe = attn_ctx.enter_context(
                    tc.tile_pool(name="pe", bufs=4))
                pool_v = attn_ctx.enter_context(
                    tc.tile_pool(name="v", bufs=PF + 2))
                pool_k = attn_ctx.enter_context(
                    tc.tile_pool(name="k", bufs=2 * (PF + 1)))
                pool_e = attn_ctx.enter_context(
                    tc.tile_pool(name="e", bufs=4))
                pool_r = attn_ctx.enter_context(
                    tc.tile_pool(name="r", bufs=4))
                # S^T tiles are [128, J, 128] fp32 = 4KB = two psum banks
                # each; one batch-pair in flight (the next pair's matmuls
                # start as soon as the first exp frees its banks).
                psum_s = attn_ctx.enter_context(
                    tc.tile_pool(name="ps", bufs=2, space="PSUM"))
                psum_pe = attn_ctx.enter_context(
                    tc.tile_pool(name="ppe", bufs=1, space="PSUM"))
                psum_av = attn_ctx.enter_context(
                    tc.tile_pool(name="pav", bufs=2, space="PSUM"))
                psum_w = attn_ctx.enter_context(
                    tc.tile_pool(name="pw", bufs=1, space="PSUM"))

                # ACT exp table load, off the critical path.
                nc.scalar.activation(
                    warm_act[:], warm_w[:, 0:16],
                    mybir.ActivationFunctionType.Exp)

                # ~2.6us of dependency-free matmuls to ramp the PE clock;
                # wt stays live as the filler target through attention.
                wt = psum_w.tile([128, 128], F32, tag="wt", name="warm_t")
                for _ in range(24):
                    nc.tensor.matmul(wt[:], warm_w[:], warm_w[:],
                                     start=True, stop=True)

                heads = {}

                def issue_head_dmas(h):
                    if h >= H:
                        return
                    qk_t = [None, None]
                    for bp in range(2):
                        t = pool_k.tile([2 * C, N + NS], F8, tag="kT",
                                        name=f"qk{h}_{bp}")
                        nc.sync.dma_start(t[:], qk_d[h, bp])
                        qk_t[bp] = t
                    pv_t = pool_v.tile([128, J, PVW], BF16, tag="vp",
                                       name=f"pv{h}")
                    nc.sync.dma_start(
                        pv_t[:], pv_d[h].rearrange("(j p) x -> p j x", p=128))
                    if 2 <= h < 2 + D // 128:
                        nc.sync.dma_start(w1_s[:, h - 2, :], w1_r[:, h - 2, :])
                    if 8 <= h < 8 + D // 128:
                        nc.sync.dma_start(w2_s[:, h - 8, :], w2_r[:, h - 8, :])
                    heads[h] = (qk_t, pv_t)

                for h in range(PF):
                    issue_head_dmas(h)

                pe4_ps = [None]

                def emit_pe4(h, parts=(0, 1)):
                    """pe @ v for all 4 batches of head h (q-major,
                    batches concatenated on the free axis).  Emitted in
                    two 4-chunk halves, one per preceding batch-pair, so
                    the PE load is even and the next S^T is never stuck
                    behind a long pe4 burst.  Returns pe4_sb after the
                    last part, else None."""
                    _, vp_t = heads[h]
                    vp4 = vp_t[:, :, NS:].rearrange(
                        "p j (b c) -> p j b c", b=B)
                    peT_t = vp_t[:, :, 0:NS]
                    if 0 in parts:
                        pe4_ps[0] = psum_pe.tile([NS, B, C + 1], F32,
                                                 tag="pe4", name="pe4")
                    pe4 = pe4_ps[0]
                    for part in parts:
                        for j in range(part * J // 2, (part + 1) * J // 2):
                            nc.tensor.matmul(
                                pe4[:], peT_t[:, j, :], vp4[:, j, :, :],
                                start=(j == 0), stop=(j == J - 1))
                    if 1 not in parts:
                        return None
                    pe4_sb = pool_pe.tile([NS, B, C + 1], F32, tag="pe4sb",
                                          name="pe4_sb")
                    # stage in SBUF: DVE may read only one PSUM input
                    nc.vector.tensor_copy(pe4_sb[:], pe4[:])
                    return pe4_sb

                def do_av(prev):
                    """AV matmuls + normalization fixup for a finished
                    batch-pair (pipelined one pair late so the PE never
                    waits on the current pair's exps)."""
                    h, bp, exps, vp_t, pe4_sb = prev
                    vp4 = vp_t[:, :, NS:].rearrange(
                        "p j (b c) -> p j b c", b=B)
                    for i in range(2):
                        b = 2 * bp + i
                        av = psum_av.tile([NS, C + 1], F32, tag="av",
                                          name="av")
                        for j in range(J):
                            nc.tensor.matmul(
                                av[:], exps[i][:, j, :], vp4[:, j, b, :],
                                start=(j == 0), stop=(j == J - 1))
                        recip = pool_r.tile([NS, 1], F32, tag="recip",
                                            name="recip")
                        nc.vector.reciprocal(recip[:], av[:, C:C + 1])
                        # x = ctx_exp/den + ctx_pe
                        nc.vector.scalar_tensor_tensor(
                            out=x_nat[b][:, h, :],
                            in0=av[:, 0:C],
                            scalar=recip[:, 0:1],
                            in1=pe4_sb[:, b, 0:C],
                            op0=mybir.AluOpType.mult,
                            op1=mybir.AluOpType.add)
                        if h % 2 == 1:
                            # both heads of chunk h//2 are in x_nat[b]:
                            # transpose to xT inline
                            t = h // 2
                            pt = psum_av.tile([128, NS], BF16, tag="av",
                                              name="pt")
                            nc.tensor.transpose(
                                pt[:], x_nat[b][:, h - 1:h + 1, :], ident[:])
                            nc.vector.tensor_copy(xT[:, t, b, :], pt[:])

                prev = None
                pe4_sb_cur = None       # pe4_sb for head h (consumed by h's
                pe4_sb_next = None      # do_av calls); next = head h+1
                for h in range(H):
                    qk_t, vp_t = heads[h]
                    if h == 0:
                        pass  # pe4(0) emitted inside (0, bp0) below
                    for bp in range(2):
                        qk = qk_t[bp]
                        exps = [None, None]
                        st = [None, None]
                        # interleaved S^T: even batch on PE rows 0:64,
                        # odd batch on rows 64:128 -> concurrent halves
                        for i in range(2):
                            st[i] = psum_s.tile([128, J, NS], F32, tag="st",
                                                name=f"st{i}")
                        for j in range(J):
                            for i in range(2):
                                s = i * C
                                nc.tensor.matmul(
                                    st[i][:, j, :],
                                    qk[s:s + C, j * 128:(j + 1) * 128],
                                    qk[s:s + C, N:],
                                    start=True, stop=True)
                        # dependency-free fillers: keep the PE activity
                        # monitor fed so the clock stays at 8/8 (the
                        # packed S^T halves array duty; HAM re-throttles
                        # an under-occupied PE even with no real gaps).
                        # LDWEIGHTS-only: no psum write, so the fillers
                        # don't contend with the ACT/DVE psum reads.
                        for _ in range(2):
                            nc.tensor.ldweights(warm_w[:])
                        for i in range(2):
                            e = pool_e.tile([128, J, NS], BF16, tag="expS",
                                            name=f"expS{i}")
                            nc.scalar.activation(
                                e[:], st[i][:],
                                mybir.ActivationFunctionType.Exp,
                                scale=SCALE)
                            exps[i] = e

                        if h == 0 and bp == 0:
                            pe4_sb_cur = emit_pe4(0)
                        if bp == 1 and h + 1 < H:
                            # pipeline next head's pe@v half a head early
                            pe4_sb_next = emit_pe4(h + 1)
                        if prev is not None:
                            do_av(prev)
                        prev = (h, bp, exps, vp_t, pe4_sb_cur)
                    pe4_sb_cur = pe4_sb_next
                    issue_head_dmas(h + PF)
                do_av(prev)

            # ---------------- MLP ----------------
            with ExitStack() as mlp_ctx:
                psum_h1 = mlp_ctx.enter_context(
                    tc.tile_pool(name="ph1", bufs=3, space="PSUM"))
                psum_y = mlp_ctx.enter_context(
                    tc.tile_pool(name="py", bufs=2, space="PSUM"))

                # b2 broadcast to all 128 row-partitions (PE outer
                # product with ones), staged to SBUF for the fc2 bias add
                b2b = const.tile([128, D], BF16, tag="b2b", name="b2b")
                for nn in range(2):
                    yb = psum_y.tile([128, 512], F32, tag="y", name="b2bp")
                    nc.tensor.matmul(
                        yb[:], ones1[:1, :], b2_s[:1, nn * 512:(nn + 1) * 512],
                        start=True, stop=True)
                    nc.vector.tensor_copy(b2b[:, nn * 512:(nn + 1) * 512],
                                          yb[:])

                # fc1: hdn^T[do, rows] = sum_i w1[i]^T.T @ xT[i]
                pool_sg = mlp_ctx.enter_context(tc.tile_pool(name="sg",
                                                             bufs=3))
                for o in range(D // 128):
                    h1 = psum_h1.tile([128, B, NS], F32, tag="h1")
                    for i in range(D // 128):
                        nc.tensor.matmul(
                            h1[:], w1_s[:, i, o * 128:(o + 1) * 128],
                            xT[:, i, :, :],
                            start=(i == 0), stop=(i == D // 128 - 1))
                    # silu(z) = z * sigmoid(z), z = h1 + b1
                    sg = pool_sg.tile([128, B, NS], F32, tag="sg")
                    nc.scalar.activation(
                        sg[:], h1[:],
                        mybir.ActivationFunctionType.Sigmoid,
                        bias=b1_s[:, o:o + 1])
                    nc.vector.scalar_tensor_tensor(
                        out=hdnT[:, o, :, :],
                        in0=h1[:],
                        scalar=b1_s[:, o:o + 1],
                        in1=sg[:],
                        op0=mybir.AluOpType.add,
                        op1=mybir.AluOpType.mult)

                # fc2: y[rows, do] = sum_i hdnT[i].T @ w2[i]  (+ b2)
                pool_o = mlp_ctx.enter_context(tc.tile_pool(name="o",
                                                            bufs=3))
                for t in range(B):
                    for nn in range(2):
                        y = psum_y.tile([128, 512], F32, tag="y")
                        for i in range(D // 128):
                            nc.tensor.matmul(
                                y[:], hdnT[:, i, t, :],
                                w2_s[:, i, nn * 512:(nn + 1) * 512],
                                start=(i == 0), stop=(i == D // 128 - 1))
                        y_sb = pool_o.tile([128, 512], F32, tag="ysb")
                        nc.vector.tensor_tensor(
                            out=y_sb[:], in0=y[:],
                            in1=b2b[:, nn * 512:(nn + 1) * 512],
                            op=mybir.AluOpType.add)
                        nc.sync.dma_start(
                            out_d[t, :, nn * 512:(nn + 1) * 512], y_sb[:])

    nc.compile()
    return nc


_PROG = None


def _get_prog():
    global _PROG
    if _PROG is None:
        _PROG = build_program()
    return _PROG


def make_in_maps(q, k, v, pe, w1, b1, w2, b2):
    import ml_dtypes
    bf = ml_dtypes.bfloat16
    f8 = ml_dtypes.float8_e4m3
    # [b,h,n,c] -> [h, b//2, (b%2)*C+c, n]
    qT = np.transpose(q, (1, 0, 3, 2)).reshape(H, B // 2, 2 * C, N)
    kT = np.transpose(k, (1, 0, 3, 2)).reshape(H, B // 2, 2 * C, N)
    vp = np.concatenate([v, np.ones((B, H, N, 1), v.dtype)], axis=-1)
    vp = np.transpose(vp, (1, 2, 0, 3)).reshape(H, N, B * (C + 1)).astype(bf)
    peT = np.transpose(pe[0], (0, 2, 1)).astype(bf)
    w1c = np.ascontiguousarray(w1).astype(bf)
    w2c = np.ascontiguousarray(w2).astype(bf)
    b1f = np.ascontiguousarray(b1).astype(np.float32)
    b2c = np.ascontiguousarray(b2).astype(bf)
    idm = np.eye(128, dtype=np.float32).astype(bf)

    in_maps = []
    for r in range(NCORES):
        sl = slice(r * NS, (r + 1) * NS)
        # kT is full N (not sharded); qT carries this core's q rows
        qk = np.ascontiguousarray(
            np.concatenate([kT, qT[:, :, :, sl]], axis=-1)).astype(f8)
        pv = np.ascontiguousarray(
            np.concatenate([peT[:, :, sl], vp], axis=-1))
        in_maps.append({
            "qk": qk,
            "pv": pv,
            "idm": idm,
            "w1s": w1c,
            "b1s": b1f,
            "w2s": w2c,
            "b2s": b2c,
        })
    return in_maps


def assemble(results):
    out = np.empty((B, N, D), np.float32)
    for r in range(NCORES):
        out[:, r * NS:(r + 1) * NS, :] = results[r]["out"]
    return out


def kernel(q, k, v, pe, w1, b1, w2, b2):
    nc = _get_prog()
    in_maps = make_in_maps(q, k, v, pe, w1, b1, w2, b2)
    res = run_bass_kernel_spmd(nc, in_maps, core_ids=list(range(NCORES)))
    return assemble(res.results)


# revision 29
# speedup vs baseline: 1.5048x; 1.0341x over previous
"""Trainium2 Bass kernel for nn_Attention_40020505264416.

Reference computation (B=4, H=16, N=1024, C=64, D=H*C=1024):
    scores = einsum('bhnc,bhmc->bhnm', q, k) * C**-0.5
    attn   = pe + softmax(scores, axis=-1)          # post-softmax bias
    ctx    = einsum('bhnm,bhmc->bhnc', attn, v)
    x      = ctx.transpose(0,2,1,3).reshape(B, N, D)
    out    = silu(x @ w1 + b1) @ w2 + b2

Distribution: pure data-parallel over query rows (N sharded 8-way, 128
rows per core).  No inter-core communication.  Host-side layouts:

  qk  [H,B/2,128,N+NS] fp8   kT|qT packed, two batches stacked on the
                             partition axis (b even rows 0:64, b odd
                             64:128) -> the two batches' S^T matmuls
                             target disjoint PE row-groups and run
                             CONCURRENTLY (tile_position auto-derived
                             from base_partition).
  pv  [H,N,NS+B*(C+1)] bf16  peT slices | v with ones column (the AV
                             matmul emits the softmax denominator as
                             psum column 64 for free)
  w*, b*                     MLP weights, natural layout

Steady state is ACT(exp)-bound: per batch-pair the ACT does two
[128,1024] exps (~2.2us) while the PE does the 16 interleaved S^T
matmuls (concurrent row-halves), the previous pair's AV + fixups, and
an eighth of the head's pe@v.  S^T psum tiles are bf16 (1 bank each) so
four tiles = two batch-pairs pipeline in 4 of the 8 psum banks.

DMA: everything rides the HWDGE rings.  sync: qk + pv + w1 (streamed
over heads 6..13) + w2 (MLP start) + outputs; scalar: tiny init loads.
Explicit 2-head prefetch keeps the PE/ACT pipe fed; q/k in fp8 keeps
the attention-window DMA demand at ~245 GB/s (<358 limit).  fp8 scores
only perturb softmax weights ~5% rel; the softmax branch contributes
~0.2% of x's variance (pe@v dominates), so the end-to-end error stays
at the bf16 baseline level.
"""

import os
import sys

for _p in ("/opt/trn_rl_repo",):
    if os.path.isdir(_p) and _p not in sys.path:
        sys.path.insert(0, _p)

import numpy as np

import concourse.bass as bass
import concourse.mybir as mybir
import concourse.tile as tile
from concourse import bacc
from concourse.bass_utils import run_bass_kernel_spmd

B, H, N, C = 4, 16, 1024, 64
D = H * C
NCORES = 8
NS = N // NCORES          # query rows per core
J = N // 128              # key chunks of 128
SCALE = C ** -0.5
PF = 2                    # DMA prefetch depth (heads)

PVW = NS + B * (C + 1)    # packed peT|v' row width
JH = J // 2               # half of the key chunks (one 1-bank psum tile)
F32 = mybir.dt.float32
BF16 = mybir.dt.bfloat16
F8 = mybir.dt.float8e4


def build_program():
    nc = bacc.Bacc(None, debug=False)

    # k^T and q^T packed in one tensor, two batches stacked on the
    # partition axis: [h, b//2, (b%2)*C+c, 0:N]=kT, [.., N:N+NS]=qT
    qk_d = nc.dram_tensor("qk", [H, B // 2, 2 * C, N + NS], F8,
                          kind="ExternalInput")
    # pe^T and v' packed per head: [h, m, 0:NS]=peT(q), [h, m, NS:]=v'(b,c+1)
    pv_d = nc.dram_tensor("pv", [H, N, PVW], BF16, kind="ExternalInput")
    idm_d = nc.dram_tensor("idm", [128, 128], BF16, kind="ExternalInput")
    w1_d = nc.dram_tensor("w1s", [D, D], BF16, kind="ExternalInput")
    b1_d = nc.dram_tensor("b1s", [D], F32, kind="ExternalInput")
    w2_d = nc.dram_tensor("w2s", [D, D], BF16, kind="ExternalInput")
    b2_d = nc.dram_tensor("b2s", [D], BF16, kind="ExternalInput")
    out_d = nc.dram_tensor("out", [B, NS, D], F32, kind="ExternalOutput")

    with tile.TileContext(nc) as tc:
        from contextlib import ExitStack

        with ExitStack() as ctx:
            const = ctx.enter_context(tc.tile_pool(name="const", bufs=1))

            ident = const.tile([128, 128], BF16, tag="ident")
            nc.scalar.dma_start(ident[:], idm_d[:])
            ones1 = const.tile([1, 128], BF16, tag="ones1")
            nc.vector.memset(ones1[:], 1.0)

            w1_s = const.tile([128, D // 128, D], BF16, tag="w1s")
            w2_s = const.tile([128, D // 128, D], BF16, tag="w2s")
            w1_r = w1_d.rearrange("(i p) o -> p i o", p=128)
            w2_r = w2_d.rearrange("(i p) o -> p i o", p=128)
            b1_s = const.tile([128, D // 128], F32, tag="b1s")
            nc.scalar.dma_start(b1_s[:], b1_d.rearrange("(o p) -> p o", p=128))
            b2_s = const.tile([1, D], BF16, tag="b2s")
            nc.scalar.dma_start(b2_s[:], b2_d.rearrange("(x d) -> x d", x=1))

            # warm-up fodder: dependency-free matmuls ramp the PE clock
            # while the first DMAs land; a dummy Exp loads the ACT table.
            warm_w = const.tile([128, 128], BF16, tag="warmw", name="warm_w")
            nc.vector.memset(warm_w[:], 0.0)
            warm_act = const.tile([128, 16], F32, tag="warma", name="warm_act")

            # Attention output, natural layout [q, d] per batch.
            x_nat = [const.tile([NS, H, C], BF16, tag=f"xnat{b}",
                                name=f"xnat{b}")
                     for b in range(B)]
            # x^T chunks [d-in-chunk, chunk, b, q] and hdn^T chunks.
            xT = const.tile([128, D // 128, B, NS], BF16, tag="xT")
            hdnT = const.tile([128, D // 128, B, NS], BF16, tag="hdnT")

            # ---------------- attention ----------------
            with ExitStack() as attn_ctx:
                pool_pe = attn_ctx.enter_context(
                    tc.tile_pool(name="pe", bufs=4))
                pool_v = attn_ctx.enter_context(
                    tc.tile_pool(name="v", bufs=PF + 2))
                pool_k = attn_ctx.enter_context(
                    tc.tile_pool(name="k", bufs=2 * (PF + 1)))
                pool_e = attn_ctx.enter_context(
                    tc.tile_pool(name="e", bufs=4))
                pool_r = attn_ctx.enter_context(
                    tc.tile_pool(name="r", bufs=4))
                # S^T tiles are [128, J, 128] fp32 = 4KB = two psum banks
                # each; one batch-pair in flight (the next pair's matmuls
                # start as soon as the first exp frees its banks).
                psum_s = attn_ctx.enter_context(
                    tc.tile_pool(name="ps", bufs=2, space="PSUM"))
                psum_pe = attn_ctx.enter_context(
                    tc.tile_pool(name="ppe", bufs=1, space="PSUM"))
                psum_av = attn_ctx.enter_context(
                    tc.tile_pool(name="pav", bufs=2, space="PSUM"))
                psum_w = attn_ctx.enter_context(
                    tc.tile_pool(name="pw", bufs=1, space="PSUM"))

                # ACT table loads (exp + sigmoid), off the critical path;
                # sigmoid last so... both resident before first real exp.
                nc.scalar.activation(
                    warm_act[:], warm_w[:, 0:16],
                    mybir.ActivationFunctionType.Sigmoid)
                nc.scalar.activation(
                    warm_act[:], warm_w[:, 0:16],
                    mybir.ActivationFunctionType.Exp)

                # ~2.6us of dependency-free matmuls to ramp the PE clock;
                # wt stays live as the filler target through attention.
                wt = psum_w.tile([128, 128], F32, tag="wt", name="warm_t")
                for _ in range(24):
                    nc.tensor.matmul(wt[:], warm_w[:], warm_w[:],
                                     start=True, stop=True)

                heads = {}

                def issue_head_dmas(h):
                    if h >= H:
                        return
                    qk_t = [None, None]
                    for bp in range(2):
                        t = pool_k.tile([2 * C, N + NS], F8, tag="kT",
                                        name=f"qk{h}_{bp}")
                        nc.sync.dma_start(t[:], qk_d[h, bp])
                        qk_t[bp] = t
                    pv_t = pool_v.tile([128, J, PVW], BF16, tag="vp",
                                       name=f"pv{h}")
                    nc.sync.dma_start(
                        pv_t[:], pv_d[h].rearrange("(j p) x -> p j x", p=128))
                    if 2 <= h < 2 + D // 128:
                        nc.sync.dma_start(w1_s[:, h - 2, :], w1_r[:, h - 2, :])
                    if 8 <= h < 8 + D // 128:
                        nc.sync.dma_start(w2_s[:, h - 8, :], w2_r[:, h - 8, :])
                    heads[h] = (qk_t, pv_t)

                for h in range(PF):
                    issue_head_dmas(h)

                pe4_ps = [None]

                def emit_pe4(h, parts=(0, 1)):
                    """pe @ v for all 4 batches of head h (q-major,
                    batches concatenated on the free axis).  Emitted in
                    two 4-chunk halves, one per preceding batch-pair, so
                    the PE load is even and the next S^T is never stuck
                    behind a long pe4 burst.  Returns pe4_sb after the
                    last part, else None."""
                    _, vp_t = heads[h]
                    vp4 = vp_t[:, :, NS:].rearrange(
                        "p j (b c) -> p j b c", b=B)
                    peT_t = vp_t[:, :, 0:NS]
                    if 0 in parts:
                        pe4_ps[0] = psum_pe.tile([NS, B, C + 1], F32,
                                                 tag="pe4", name="pe4")
                    pe4 = pe4_ps[0]
                    for part in parts:
                        for j in range(part * J // 2, (part + 1) * J // 2):
                            nc.tensor.matmul(
                                pe4[:], peT_t[:, j, :], vp4[:, j, :, :],
                                start=(j == 0), stop=(j == J - 1))
                    if 1 not in parts:
                        return None
                    pe4_sb = pool_pe.tile([NS, B, C + 1], F32, tag="pe4sb",
                                          name="pe4_sb")
                    # stage in SBUF: DVE may read only one PSUM input
                    nc.vector.tensor_copy(pe4_sb[:], pe4[:])
                    return pe4_sb

                def do_av(prev):
                    """AV matmuls + normalization fixup for a finished
                    batch-pair (pipelined one pair late so the PE never
                    waits on the current pair's exps)."""
                    h, bp, exps, vp_t, pe4_sb = prev
                    vp4 = vp_t[:, :, NS:].rearrange(
                        "p j (b c) -> p j b c", b=B)
                    for i in range(2):
                        b = 2 * bp + i
                        av = psum_av.tile([NS, C + 1], F32, tag="av",
                                          name="av")
                        for j in range(J):
                            nc.tensor.matmul(
                                av[:], exps[i][:, j, :], vp4[:, j, b, :],
                                start=(j == 0), stop=(j == J - 1))
                        recip = pool_r.tile([NS, 1], F32, tag="recip",
                                            name="recip")
                        nc.vector.reciprocal(recip[:], av[:, C:C + 1])
                        # x = ctx_exp/den + ctx_pe
                        nc.vector.scalar_tensor_tensor(
                            out=x_nat[b][:, h, :],
                            in0=av[:, 0:C],
                            scalar=recip[:, 0:1],
                            in1=pe4_sb[:, b, 0:C],
                            op0=mybir.AluOpType.mult,
                            op1=mybir.AluOpType.add)
                        if h % 2 == 1:
                            # both heads of chunk h//2 are in x_nat[b]:
                            # transpose to xT inline
                            t = h // 2
                            pt = psum_av.tile([128, NS], BF16, tag="av",
                                              name="pt")
                            nc.tensor.transpose(
                                pt[:], x_nat[b][:, h - 1:h + 1, :], ident[:])
                            nc.vector.tensor_copy(xT[:, t, b, :], pt[:])

                prev = None
                pe4_sb_cur = None       # pe4_sb for head h (consumed by h's
                pe4_sb_next = None      # do_av calls); next = head h+1
                for h in range(H):
                    qk_t, vp_t = heads[h]
                    if h == 0:
                        pass  # pe4(0) emitted inside (0, bp0) below
                    for bp in range(2):
                        qk = qk_t[bp]
                        exps = [None, None]
                        st = [None, None]
                        # interleaved S^T: even batch on PE rows 0:64,
                        # odd batch on rows 64:128 -> concurrent halves
                        for i in range(2):
                            st[i] = psum_s.tile([128, J, NS], F32, tag="st",
                                                name=f"st{i}")
                        for j in range(J):
                            for i in range(2):
                                s = i * C
                                nc.tensor.matmul(
                                    st[i][:, j, :],
                                    qk[s:s + C, j * 128:(j + 1) * 128],
                                    qk[s:s + C, N:],
                                    start=True, stop=True)
                        # dependency-free fillers: keep the PE activity
                        # monitor fed so the clock stays at 8/8 (the
                        # packed S^T halves array duty; HAM re-throttles
                        # an under-occupied PE even with no real gaps).
                        # LDWEIGHTS-only: no psum write, so the fillers
                        # don't contend with the ACT/DVE psum reads.
                        # The first heads are DMA-gated (low real duty),
                        # so they get full matmul fillers on zeros.
                        if h < 3:
                            for _ in range(6):
                                nc.tensor.matmul(wt[:], warm_w[:],
                                                 warm_w[:],
                                                 start=True, stop=True)
                        for _ in range(2):
                            nc.tensor.ldweights(warm_w[:])
                        for i in range(2):
                            e = pool_e.tile([128, J, NS], BF16, tag="expS",
                                            name=f"expS{i}")
                            nc.scalar.activation(
                                e[:], st[i][:],
                                mybir.ActivationFunctionType.Exp,
                                scale=SCALE)
                            exps[i] = e

                        if h == 0 and bp == 0:
                            pe4_sb_cur = emit_pe4(0)
                        if bp == 1 and h + 1 < H:
                            # pipeline next head's pe@v half a head early
                            pe4_sb_next = emit_pe4(h + 1)
                        if prev is not None:
                            do_av(prev)
                        prev = (h, bp, exps, vp_t, pe4_sb_cur)
                    pe4_sb_cur = pe4_sb_next
                    issue_head_dmas(h + PF)
                do_av(prev)

            # ---------------- MLP ----------------
            with ExitStack() as mlp_ctx:
                psum_h1 = mlp_ctx.enter_context(
                    tc.tile_pool(name="ph1", bufs=3, space="PSUM"))
                psum_y = mlp_ctx.enter_context(
                    tc.tile_pool(name="py", bufs=2, space="PSUM"))

                # b2 broadcast to all 128 row-partitions (PE outer
                # product with ones), staged to SBUF for the fc2 bias add
                b2b = const.tile([128, D], BF16, tag="b2b", name="b2b")
                for nn in range(2):
                    yb = psum_y.tile([128, 512], F32, tag="y", name="b2bp")
                    nc.tensor.matmul(
                        yb[:], ones1[:1, :], b2_s[:1, nn * 512:(nn + 1) * 512],
                        start=True, stop=True)
                    nc.vector.tensor_copy(b2b[:, nn * 512:(nn + 1) * 512],
                                          yb[:])

                # fc1: hdn^T[do, rows] = sum_i w1[i]^T.T @ xT[i]
                pool_sg = mlp_ctx.enter_context(tc.tile_pool(name="sg",
                                                             bufs=3))
                for o in range(D // 128):
                    h1 = psum_h1.tile([128, B, NS], F32, tag="h1")
                    for i in range(D // 128):
                        nc.tensor.matmul(
                            h1[:], w1_s[:, i, o * 128:(o + 1) * 128],
                            xT[:, i, :, :],
                            start=(i == 0), stop=(i == D // 128 - 1))
                    # silu(z) = z * sigmoid(z), z = h1 + b1
                    sg = pool_sg.tile([128, B, NS], F32, tag="sg")
                    nc.scalar.activation(
                        sg[:], h1[:],
                        mybir.ActivationFunctionType.Sigmoid,
                        bias=b1_s[:, o:o + 1])
                    nc.vector.scalar_tensor_tensor(
                        out=hdnT[:, o, :, :],
                        in0=h1[:],
                        scalar=b1_s[:, o:o + 1],
                        in1=sg[:],
                        op0=mybir.AluOpType.add,
                        op1=mybir.AluOpType.mult)

                # fc2: y[rows, do] = sum_i hdnT[i].T @ w2[i]  (+ b2)
                pool_o = mlp_ctx.enter_context(tc.tile_pool(name="o",
                                                            bufs=3))
                for t in range(B):
                    for nn in range(2):
                        y = psum_y.tile([128, 512], F32, tag="y")
                        for i in range(D // 128):
                            nc.tensor.matmul(
                                y[:], hdnT[:, i, t, :],
                                w2_s[:, i, nn * 512:(nn + 1) * 512],
                                start=(i == 0), stop=(i == D // 128 - 1))
                        y_sb = pool_o.tile([128, 512], F32, tag="ysb")
                        nc.vector.tensor_tensor(
                            out=y_sb[:], in0=y[:],
                            in1=b2b[:, nn * 512:(nn + 1) * 512],
                            op=mybir.AluOpType.add)
                        nc.sync.dma_start(
                            out_d[t, :, nn * 512:(nn + 1) * 512], y_sb[:])

    nc.compile()
    return nc


_PROG = None


def _get_prog():
    global _PROG
    if _PROG is None:
        _PROG = build_program()
    return _PROG


def make_in_maps(q, k, v, pe, w1, b1, w2, b2):
    import ml_dtypes
    bf = ml_dtypes.bfloat16
    f8 = ml_dtypes.float8_e4m3
    # [b,h,n,c] -> [h, b//2, (b%2)*C+c, n]
    qT = np.transpose(q, (1, 0, 3, 2)).reshape(H, B // 2, 2 * C, N)
    kT = np.transpose(k, (1, 0, 3, 2)).reshape(H, B // 2, 2 * C, N)
    vp = np.concatenate([v, np.ones((B, H, N, 1), v.dtype)], axis=-1)
    vp = np.transpose(vp, (1, 2, 0, 3)).reshape(H, N, B * (C + 1)).astype(bf)
    peT = np.transpose(pe[0], (0, 2, 1)).astype(bf)
    w1c = np.ascontiguousarray(w1).astype(bf)
    w2c = np.ascontiguousarray(w2).astype(bf)
    b1f = np.ascontiguousarray(b1).astype(np.float32)
    b2c = np.ascontiguousarray(b2).astype(bf)
    idm = np.eye(128, dtype=np.float32).astype(bf)

    in_maps = []
    for r in range(NCORES):
        sl = slice(r * NS, (r + 1) * NS)
        # kT is full N (not sharded); qT carries this core's q rows
        qk = np.ascontiguousarray(
            np.concatenate([kT, qT[:, :, :, sl]], axis=-1)).astype(f8)
        pv = np.ascontiguousarray(
            np.concatenate([peT[:, :, sl], vp], axis=-1))
        in_maps.append({
            "qk": qk,
            "pv": pv,
            "idm": idm,
            "w1s": w1c,
            "b1s": b1f,
            "w2s": w2c,
            "b2s": b2c,
        })
    return in_maps


def assemble(results):
    out = np.empty((B, N, D), np.float32)
    for r in range(NCORES):
        out[:, r * NS:(r + 1) * NS, :] = results[r]["out"]
    return out


def kernel(q, k, v, pe, w1, b1, w2, b2):
    nc = _get_prog()
    in_maps = make_in_maps(q, k, v, pe, w1, b1, w2, b2)
    res = run_bass_kernel_spmd(nc, in_maps, core_ids=list(range(NCORES)))
    return assemble(res.results)
